# revision 11
# baseline (speedup 1.0000x reference)
"""Trainium2 Bass kernel for nn_ATTBase (multi-agent attention encoder/critic).

Self-contained: hardcodes shapes for B=4096, OBS=221, H=256, 8 agents
(4 adv + 4 good), 9 boxes, 2 ramps, data-parallel over 8 NeuronCores.

Pipeline per core (512 batch rows -> 4608 encoder rows -> 36 tiles of 128):
  - entity embeddings via small-K matmuls (inputs pre-transposed on host into
    32-aligned "banks" so the 4 matmuls of a group use distinct PE row-groups)
  - tanh on ScalarE straight out of PSUM (bias rides the matmul via a ones row)
  - attention (beta/softmax/weighted-sum) on VectorE in batch-on-partition
    layout using fused tensor_tensor_reduce / scalar_tensor_tensor ops
  - enc1/enc2 and the critic head as K=128 accumulation chains, with PE-based
    128x128 transposes to produce the transposed activations each matmul needs
"""
import sys
import types
import numpy as np
import ml_dtypes

H = 256
OBS = 221
B = 4096
NCORES = 8
BC = B // NCORES            # 512 batch rows per core
AGENT = 8
RROWS = BC * 9              # 4608 encoder rows per core
NT = RROWS // 128           # 36 tiles
NFT = NT - 4                # f_all tiles (encoder outputs kept on-chip)

# entity table: (weight_key, bias_key, width, col0)
_ENTS = (
    [("self_W", "self_b", 10, 0)]
    + [("seeker_W", "seeker_b", 10, 10 + 10 * j) for j in range(3)]
    + [("hider_W", "hider_b", 10, 40 + 10 * j) for j in range(4)]
    + [("box_W", "box_b", 12, 80 + 12 * j) for j in range(9)]
    + [("ramp_W", "ramp_b", 12, 197 + 12 * j) for j in range(2)]
)
N_ENT = len(_ENTS)          # 19
N_BANK = 5                  # 4 entity slots per bank, 32 partitions each
# attention blocks: (corr_key, first_entity, n_entities)
_BLOCKS = [("adv_corr", 1, 3), ("good_corr", 4, 4), ("box_corr", 8, 9), ("ramp_corr", 17, 2)]

_BF = np.float16
_CACHE = {}


def _install_hook():
    import antenv  # noqa: F401
    if "antenv.axon_hooks" in sys.modules:
        return
    mod = types.ModuleType("antenv.axon_hooks")
    state = {"hook": None}
    mod.set_axon_ntff_profile_hook = lambda h: state.__setitem__("hook", h)
    mod.get_axon_ntff_profile_hook = lambda: state["hook"]
    sys.modules["antenv.axon_hooks"] = mod
    try:
        from trn_agent_boot.trn_boot import _ntff_profile_via_ctypes
        mod.set_axon_ntff_profile_hook(
            _ntff_profile_via_ctypes('/opt/axon/libaxon_pjrt.so'))
    except Exception:
        pass


def _build_program(trace_dtype="bf16"):
    import concourse.bacc as bacc
    import concourse.tile as tile
    from concourse import mybir

    F32 = mybir.dt.float32
    F16 = mybir.dt.float16
    AF = mybir.ActivationFunctionType
    MULT = mybir.AluOpType.mult
    ADD = mybir.AluOpType.add

    nc = bacc.Bacc()
    # per-core inputs
    xt_d = nc.dram_tensor("xt", [N_BANK, NT, 128, 128], F16, kind="ExternalInput")
    # replicated params
    wemb_d = nc.dram_tensor("wemb", [2, N_BANK, 128, 256], F16, kind="ExternalInput")
    qcw_d = nc.dram_tensor("qcw", [2, 5, 2, 128, 256], F16, kind="ExternalInput")
    fcw_d = nc.dram_tensor("fcw", [2, 2, 2, 128, 128], F16, kind="ExternalInput")
    fcb_d = nc.dram_tensor("fcb", [2, 2, 128, 1], F32, kind="ExternalInput")
    e1w_d = nc.dram_tensor("e1w", [2, 10, 128, 256], F16, kind="ExternalInput")
    e2w_d = nc.dram_tensor("e2w", [2, 2, 128, 256], F16, kind="ExternalInput")
    corw_d = nc.dram_tensor("corw", [2, 128, 256], F16, kind="ExternalInput")
    cfcw_d = nc.dram_tensor("cfcw", [2, 2, 128, 128], F16, kind="ExternalInput")
    cfcb_d = nc.dram_tensor("cfcb", [2, 128, 1], F32, kind="ExternalInput")
    c1w_d = nc.dram_tensor("c1w", [4, 128, 256], F16, kind="ExternalInput")
    c2w_d = nc.dram_tensor("c2w", [2, 128, 1], F16, kind="ExternalInput")
    c2b_d = nc.dram_tensor("c2b", [128, 1], F32, kind="ExternalInput")
    iden_d = nc.dram_tensor("iden", [128, 128], F16, kind="ExternalInput")
    # outputs
    ha_d = nc.dram_tensor("ha", [BC, 256], F32, kind="ExternalOutput")
    val_d = nc.dram_tensor("val", [BC, 1], F32, kind="ExternalOutput")

    with tile.TileContext(nc) as tc:
        with (
            tc.tile_pool(name="const", bufs=1) as cp,
            tc.tile_pool(name="xt", bufs=3) as xp,
            tc.tile_pool(name="emb", bufs=2) as ep,
            tc.tile_pool(name="small", bufs=2) as sp,
            tc.tile_pool(name="work", bufs=2) as wkp,
            tc.tile_pool(name="psb", bufs=2, space="PSUM") as psb,
            tc.tile_pool(name="psm", bufs=2, space="PSUM") as psm,
            tc.tile_pool(name="pst", bufs=2, space="PSUM") as pst,
        ):
            # ---- constants ----
            wemb = [[cp.tile([128, 256], F16, name=f"wemb{s}{b}") for b in range(N_BANK)] for s in range(2)]
            qcw = [[[cp.tile([128, 256], F16, name=f"qcw{s}{j}{k}") for k in range(2)] for j in range(5)] for s in range(2)]
            fcw = [[[cp.tile([128, 128], F16, name=f"fcw{s}{k}{m}") for m in range(2)] for k in range(2)] for s in range(2)]
            fcb = [[cp.tile([128, 1], F32, name=f"fcb{s}{m}") for m in range(2)] for s in range(2)]
            e1w = [[cp.tile([128, 256], F16, name=f"e1w{s}{k}") for k in range(10)] for s in range(2)]
            e2w = [[cp.tile([128, 256], F16, name=f"e2w{s}{k}") for k in range(2)] for s in range(2)]
            corw = [cp.tile([128, 256], F16, name=f"corw{k}") for k in range(2)]
            cfcw = [[cp.tile([128, 128], F16, name=f"cfcw{k}{m}") for m in range(2)] for k in range(2)]
            cfcb = [cp.tile([128, 1], F32, name=f"cfcb{m}") for m in range(2)]
            c1w = [cp.tile([128, 256], F16, name=f"c1w{k}") for k in range(4)]
            c2w = [cp.tile([128, 1], F16, name=f"c2w{k}") for k in range(2)]
            c2b = cp.tile([128, 1], F32, name="c2b")
            iden = cp.tile([128, 128], F16, name="iden")
            f_all = cp.tile([128, NFT, 256], F16, name="f_all")

            for s in range(2):
                for b in range(N_BANK):
                    nc.sync.dma_start(wemb[s][b][:], wemb_d[s, b])
                for j in range(5):
                    for k in range(2):
                        nc.sync.dma_start(qcw[s][j][k][:], qcw_d[s, j, k])
                for k in range(2):
                    for m in range(2):
                        nc.sync.dma_start(fcw[s][k][m][:], fcw_d[s, k, m])
                    nc.sync.dma_start(fcb[s][k][:], fcb_d[s, k])
                for k in range(10):
                    nc.sync.dma_start(e1w[s][k][:], e1w_d[s, k])
                for k in range(2):
                    nc.sync.dma_start(e2w[s][k][:], e2w_d[s, k])
            for k in range(2):
                nc.sync.dma_start(corw[k][:], corw_d[k])
                nc.sync.dma_start(c2w[k][:], c2w_d[k])
                for m in range(2):
                    nc.sync.dma_start(cfcw[k][m][:], cfcw_d[k, m])
            for k in range(4):
                nc.sync.dma_start(c1w[k][:], c1w_d[k])
            for m in range(2):
                nc.sync.dma_start(cfcb[m][:], cfcb_d[m])
            nc.sync.dma_start(c2b[:], c2b_d[:])
            nc.sync.dma_start(iden[:], iden_d[:])

            def transpose_pair(src_tile, dst_ap, tag):
                """src [128, 256] bf16 -> dst [128, 2, 128] (dst_ap selects slices)."""
                pt = pst.tile([128, 2, 128], F16, tag="pst", name=f"pt_{tag}")
                nc.tensor.transpose(pt[:, 0, :], src_tile[:, 0:128], iden[:])
                nc.tensor.transpose(pt[:, 1, :], src_tile[:, 128:256], iden[:])
                nc.any.tensor_copy(dst_ap, pt[:])

            def attention(emb3_fn, qc_idx_fn, blocks, tag):
                """Generic attention: returns list of vi fp16 tiles per block.
                emb3_fn(bi) -> [128, n, 256] fp16 AP of the block's entity embeddings,
                qc_idx_fn(bi) -> [128, 256] fp16 qc AP for block bi."""
                nblk = len(blocks)
                beta = wkp.tile([128, nblk, 9], F32, tag="beta", name=f"beta_{tag}")
                ex = wkp.tile([128, nblk, 9], F32, tag="ex", name=f"ex_{tag}")
                den = wkp.tile([128, nblk], F32, tag="den", name=f"den_{tag}")
                rden = wkp.tile([128, nblk], F32, tag="rden", name=f"rden_{tag}")
                vis = []
                for bi, (e0, n) in enumerate(blocks):
                    # beta via block-wide product (2x mode) + fp16 add-tree + reduce
                    emb3 = emb3_fn(bi)
                    qcb = qc_idx_fn(bi).unsqueeze(1).broadcast_to([128, n, 256])
                    prod = wkp.tile([128, n, 256], F16, tag=f"prod{bi}",
                                    name=f"prod_{tag}_{bi}")
                    nc.vector.tensor_tensor(prod[:], emb3, qcb, MULT)
                    t1 = wkp.tile([128, n, 128], F16, tag=f"t1_{bi}",
                                  name=f"t1_{tag}_{bi}")
                    nc.vector.tensor_tensor(t1[:], prod[:, :, 0:128],
                                            prod[:, :, 128:256], ADD)
                    t2 = wkp.tile([128, n, 64], F16, tag=f"t2_{bi}",
                                  name=f"t2_{tag}_{bi}")
                    nc.vector.tensor_tensor(t2[:], t1[:, :, 0:64], t1[:, :, 64:128], ADD)
                    nc.vector.tensor_reduce(beta[:, bi, 0:n], t2[:],
                                            mybir.AxisListType.X, ADD)
                    nc.scalar.activation(ex[:, bi, 0:n], beta[:, bi, 0:n], AF.Exp,
                                         accum_out=den[:, bi:bi + 1])
                nc.vector.reciprocal(rden[:], den[:])
                for bi, (e0, n) in enumerate(blocks):
                    emb3 = emb3_fn(bi)
                    # weighted sum: f32 running tile (unnormalized exp overflows fp16)
                    va = wkp.tile([128, 256], F32, tag="vina", name=f"vina_{tag}_{bi}")
                    vb = wkp.tile([128, 256], F32, tag="vinb", name=f"vinb_{tag}_{bi}")
                    cur, nxt = va, vb
                    nc.vector.tensor_scalar_mul(cur[:], emb3[:, 0, :], ex[:, bi, 0:1])
                    for j in range(1, n):
                        nc.vector.scalar_tensor_tensor(
                            out=nxt[:], in0=emb3[:, j, :], scalar=ex[:, bi, j:j + 1],
                            in1=cur[:], op0=MULT, op1=ADD)
                        cur, nxt = nxt, cur
                    vi = wkp.tile([128, 256], F16, tag=f"vi{bi}", name=f"vi_{tag}_{bi}")
                    nc.vector.tensor_scalar_mul(vi[:], cur[:], rden[:, bi:bi + 1])
                    vis.append(vi)
                return vis

            # ================= encoder tiles =================
            for t in range(NT):
                s = 0 if t < 4 else 1
                xts = [xp.tile([128, 128], F16, tag=f"x{b}", name=f"x{b}_{t}") for b in range(N_BANK)]
                for b in range(N_BANK):
                    nc.sync.dma_start(xts[b][:], xt_d[b, t])

                emb = ep.tile([128, N_ENT * 256], F16, tag="emb", name=f"emb_{t}")
                # one PSUM bank per entity: concurrent row-group matmuls
                # sharing a bank trip the HW collision detector
                for g in range(10):
                    ents = list(range(2 * g, min(2 * g + 2, N_ENT)))
                    pt = psb.tile([128, 2, 512], F32, tag="psb", name=f"embps_{t}_{g}")
                    for q, e in enumerate(ents):
                        b, sl = divmod(e, 4)
                        off = 32 * sl
                        w = _ENTS[e][2] + 1  # +1 ones row (bias)
                        nc.tensor.matmul(pt[:, q, 0:256],
                                         xts[b][off:off + w, :],
                                         wemb[s][b][off:off + w, :],
                                         start=True, stop=True,
                                         tile_position=(off, 0))
                    ne = len(ents)
                    nc.scalar.activation(
                        emb[:, ents[0] * 256:(ents[0] + ne) * 256].rearrange(
                            "p (q n) -> p q n", q=ne),
                        pt[:, 0:ne, 0:256], AF.Tanh)

                # emb_self transpose -> esT
                esT = sp.tile([128, 2, 128], F16, tag="esT", name=f"esT_{t}")
                transpose_pair(emb, esT[:], f"es_{t}")

                # qc for 4 blocks (PSUM) then to SBUF bf16
                qc_ps = psb.tile([128, 1024], F32, tag="psb", name=f"qcps_{t}")
                for j in range(4):
                    for k in range(2):
                        nc.tensor.matmul(qc_ps[:, j * 256:(j + 1) * 256],
                                         esT[:, k, :], qcw[s][j][k][:],
                                         start=(k == 0), stop=(k == 1))
                qc = sp.tile([128, 4, 256], F16, tag="qc", name=f"qc_{t}")
                nc.any.tensor_copy(qc[:, 0:2, :], qc_ps[:, 0:512].rearrange("p (j n) -> p j n", j=2))
                nc.any.tensor_copy(qc[:, 2:4, :], qc_ps[:, 512:1024].rearrange("p (j n) -> p j n", j=2))

                # giT (orientation A: lhsT = fc_W chunk, rhs = esT) + tanh w/ bias
                catT = sp.tile([128, 10, 128], F16, tag="catT", name=f"catT_{t}")
                gi_ps = psm.tile([128, 256], F32, tag="psm", name=f"gips_{t}")
                for m in range(2):
                    for k in range(2):
                        nc.tensor.matmul(gi_ps[:, m * 128:(m + 1) * 128],
                                         fcw[s][k][m][:], esT[:, k, :],
                                         start=(k == 0), stop=(k == 1))
                    nc.scalar.activation(catT[:, m, :], gi_ps[:, m * 128:(m + 1) * 128],
                                         AF.Tanh, bias=fcb[s][m][:])

                # attention over 4 blocks
                emb3v = emb[:].rearrange("p (e n) -> p e n", e=N_ENT)
                vis = attention(
                    emb3_fn=lambda bi: emb3v[:, _BLOCKS[bi][1]:_BLOCKS[bi][1] + _BLOCKS[bi][2], :],
                    qc_idx_fn=lambda b: qc[:, b, :],
                    blocks=[(e0, n) for (_, e0, n) in _BLOCKS], tag=f"t{t}")
                for bi in range(4):
                    transpose_pair(vis[bi], catT[:, 2 + 2 * bi:4 + 2 * bi, :], f"vi{bi}_{t}")

                # enc1: h = tanh(cat @ E1)
                h_ps = psm.tile([128, 256], F32, tag="psm", name=f"hps_{t}")
                for kc in range(10):
                    nc.tensor.matmul(h_ps[:], catT[:, kc, :], e1w[s][kc][:],
                                     start=(kc == 0), stop=(kc == 9))
                h = sp.tile([128, 256], F16, tag="h", name=f"h_{t}")
                nc.scalar.activation(h[:], h_ps[:], AF.Tanh)
                hT = sp.tile([128, 2, 128], F16, tag="hT", name=f"hT_{t}")
                transpose_pair(h, hT[:], f"h_{t}")

                # enc2: f = tanh(h @ E2)
                f_ps = psm.tile([128, 256], F32, tag="psm", name=f"fps_{t}")
                for k in range(2):
                    nc.tensor.matmul(f_ps[:], hT[:, k, :], e2w[s][k][:],
                                     start=(k == 0), stop=(k == 1))
                if t < 4:
                    fo = sp.tile([128, 256], F32, tag="fo", name=f"fo_{t}")
                    nc.scalar.activation(fo[:], f_ps[:], AF.Tanh)
                    nc.sync.dma_start(ha_d[t * 128:(t + 1) * 128, :], fo[:])
                else:
                    nc.scalar.activation(f_all[:, t - 4, :], f_ps[:], AF.Tanh)

            # ================= critic tiles =================
            for t in range(4):
                fiT = sp.tile([128, 2, 128], F16, tag="fiT", name=f"fiT_{t}")
                transpose_pair(f_all[:, t, :], fiT[:], f"fi_{t}")

                qcc_ps = psm.tile([128, 256], F32, tag="psm", name=f"qccps_{t}")
                for k in range(2):
                    nc.tensor.matmul(qcc_ps[:], fiT[:, k, :], corw[k][:],
                                     start=(k == 0), stop=(k == 1))
                qcc = sp.tile([128, 256], F16, tag="qcc", name=f"qcc_{t}")
                nc.any.tensor_copy(qcc[:], qcc_ps[:])

                cat2T = sp.tile([128, 4, 128], F16, tag="cat2T", name=f"cat2T_{t}")
                gic_ps = psm.tile([128, 256], F32, tag="psm", name=f"gicps_{t}")
                for m in range(2):
                    for k in range(2):
                        nc.tensor.matmul(gic_ps[:, m * 128:(m + 1) * 128],
                                         cfcw[k][m][:], fiT[:, k, :],
                                         start=(k == 0), stop=(k == 1))
                    nc.scalar.activation(cat2T[:, m, :], gic_ps[:, m * 128:(m + 1) * 128],
                                         AF.Tanh, bias=cfcb[m][:])

                fij3 = f_all[:, 4 + t::4, :]  # [128, 7, 256] strided view
                vic = attention(
                    emb3_fn=lambda bi: fij3,
                    qc_idx_fn=lambda b: qcc[:],
                    blocks=[(0, 7)], tag=f"c{t}")[0]
                transpose_pair(vic, cat2T[:, 2:4, :], f"vic_{t}")

                hc_ps = psm.tile([128, 256], F32, tag="psm", name=f"hcps_{t}")
                for kc in range(4):
                    nc.tensor.matmul(hc_ps[:], cat2T[:, kc, :], c1w[kc][:],
                                     start=(kc == 0), stop=(kc == 3))
                hc = sp.tile([128, 256], F16, tag="hc", name=f"hc_{t}")
                nc.scalar.activation(hc[:], hc_ps[:], AF.Tanh)
                hcT = sp.tile([128, 2, 128], F16, tag="hcT", name=f"hcT_{t}")
                transpose_pair(hc, hcT[:], f"hc_{t}")

                val_ps = psm.tile([128, 256], F32, tag="psm", name=f"valps_{t}")
                for k in range(2):
                    nc.tensor.matmul(val_ps[:, 0:1], hcT[:, k, :], c2w[k][:],
                                     start=(k == 0), stop=(k == 1))
                valo = sp.tile([128, 1], F32, tag="valo", name=f"valo_{t}")
                nc.scalar.activation(valo[:], val_ps[:, 0:1], AF.Identity, bias=c2b[:])
                nc.sync.dma_start(val_d[t * 128:(t + 1) * 128, :], valo[:])

    nc.finalize()
    return nc


def _pack_params(actor_params, encoder_params, correlation_mat, fc_W, fc_b,
                 critic1_W, critic1_b, critic2_W, critic2_b):
    """Host-side packing of all weights into the DRAM layouts the program expects."""
    psets = [actor_params, encoder_params]
    wemb = np.zeros((2, N_BANK, 128, 256), np.float32)
    qcw = np.zeros((2, 5, 2, 128, 256), np.float32)
    fcw = np.zeros((2, 2, 2, 128, 128), np.float32)
    fcb = np.zeros((2, 2, 128, 1), np.float32)
    e1w = np.zeros((2, 10, 128, 256), np.float32)
    e2w = np.zeros((2, 2, 128, 256), np.float32)
    for s, p in enumerate(psets):
        for e, (wk, bk, w, _c0) in enumerate(_ENTS):
            b, sl = divmod(e, 4)
            off = 32 * sl
            wemb[s, b, off:off + w, :] = np.asarray(p[wk], np.float32)
            wemb[s, b, off + w, :] = np.asarray(p[bk], np.float32)
        corrs = [p["adv_corr"], p["good_corr"], p["box_corr"], p["ramp_corr"], p["fc_W"]]
        for j, c in enumerate(corrs):
            c = np.asarray(c, np.float32)
            for k in range(2):
                qcw[s, j, k] = c[128 * k:128 * (k + 1), :]
        fw = np.asarray(p["fc_W"], np.float32)
        for k in range(2):
            for m in range(2):
                fcw[s, k, m] = fw[128 * k:128 * (k + 1), 128 * m:128 * (m + 1)]
        fb = np.asarray(p["fc_b"], np.float32)
        for m in range(2):
            fcb[s, m, :, 0] = fb[128 * m:128 * (m + 1)]
        E1 = np.asarray(p["enc1_W"], np.float32)
        for k in range(10):
            e1w[s, k] = E1[128 * k:128 * (k + 1), :]
        E2 = np.asarray(p["enc2_W"], np.float32)
        for k in range(2):
            e2w[s, k] = E2[128 * k:128 * (k + 1), :]
        # enc1_b / enc2_b are zeros in this model; assert to be safe
        assert not np.any(np.asarray(p["enc1_b"])), "nonzero enc1_b unsupported"
        assert not np.any(np.asarray(p["enc2_b"])), "nonzero enc2_b unsupported"

    corw = np.stack([np.asarray(correlation_mat, np.float32)[128 * k:128 * (k + 1), :]
                     for k in range(2)])
    fw = np.asarray(fc_W, np.float32)
    cfcw = np.zeros((2, 2, 128, 128), np.float32)
    for k in range(2):
        for m in range(2):
            cfcw[k, m] = fw[128 * k:128 * (k + 1), 128 * m:128 * (m + 1)]
    cfcb = np.asarray(fc_b, np.float32).reshape(2, 128, 1)
    C1 = np.asarray(critic1_W, np.float32)
    assert not np.any(np.asarray(critic1_b)), "nonzero critic1_b unsupported"
    c1w = np.stack([C1[128 * k:128 * (k + 1), :] for k in range(4)])
    C2 = np.asarray(critic2_W, np.float32)
    c2w = np.stack([C2[128 * k:128 * (k + 1), :] for k in range(2)])
    c2b = np.full((128, 1), float(np.asarray(critic2_b).reshape(-1)[0]), np.float32)

    return {
        "wemb": wemb.astype(_BF), "qcw": qcw.astype(_BF), "fcw": fcw.astype(_BF),
        "fcb": fcb, "e1w": e1w.astype(_BF), "e2w": e2w.astype(_BF),
        "corw": corw.astype(_BF), "cfcw": cfcw.astype(_BF), "cfcb": cfcb,
        "c1w": c1w.astype(_BF), "c2w": c2w.astype(_BF), "c2b": c2b,
        "iden": np.eye(128, dtype=np.float32).astype(_BF),
    }


def _pack_xt(x_rows):
    """x_rows [RROWS, OBS] f32 -> XT [N_BANK, NT, 128, 128] bf16 (transposed,
    entity features at 32-aligned slots, ones row for the bias)."""
    xt = np.zeros((N_BANK, NT, 128, 128), np.float32)
    xr = x_rows.reshape(NT, 128, OBS)
    for e, (_wk, _bk, w, c0) in enumerate(_ENTS):
        b, sl = divmod(e, 4)
        off = 32 * sl
        blk = xr[:, :, c0:c0 + w]                      # [NT, 128, w]
        xt[b, :, off:off + w, :] = blk.transpose(0, 2, 1)
        xt[b, :, off + w, :] = 1.0
    return xt.astype(_BF)


def kernel(share_inputs, inputs, rnn_hxs, masks, actor_params, encoder_params,
           correlation_mat, fc_W, fc_b, critic1_W, critic1_b, critic2_W, critic2_b,
           agent_i, agent_num, adv_num, good_num, box_num, ramp_num):
    from concourse.bass_utils import run_bass_kernel_spmd

    share_inputs = np.asarray(share_inputs, np.float32)
    inputs = np.asarray(inputs, np.float32)
    agent_i = int(agent_i)
    agent_num = int(agent_num)
    assert (int(adv_num), int(good_num), int(box_num), int(ramp_num)) == (4, 4, 9, 2)
    assert inputs.shape == (B, OBS) and share_inputs.shape == (B, OBS * agent_num)

    if "prog" not in _CACHE:
        _CACHE["prog"] = _build_program()
    nc = _CACHE["prog"]

    params = _pack_params(actor_params, encoder_params, correlation_mat, fc_W, fc_b,
                          critic1_W, critic1_b, critic2_W, critic2_b)

    idx = np.array([i for i in range(agent_num) if i != agent_i])
    share3 = share_inputs.reshape(B, agent_num, OBS)
    in_maps = []
    for c in range(NCORES):
        rows = slice(BC * c, BC * (c + 1))
        A = inputs[rows]
        others = share3[rows][:, idx]                   # [BC, 7, OBS]
        x_rows = np.concatenate(
            [A, A, others.transpose(1, 0, 2).reshape(BC * 7, OBS)], axis=0)
        m = dict(params)
        m["xt"] = _pack_xt(x_rows)
        in_maps.append(m)

    import os
    trace = bool(os.environ.get("KERNEL_TRACE"))
    if trace:
        _install_hook()
    res = run_bass_kernel_spmd(nc, in_maps, list(range(NCORES)), trace=trace)
    global LAST_EXEC_NS, LAST_PROFILE
    LAST_EXEC_NS = res.exec_time_ns
    LAST_PROFILE = res.profile_json
    value = np.concatenate([res.results[c]["val"] for c in range(NCORES)], axis=0)
    hidden_actor = np.concatenate([res.results[c]["ha"] for c in range(NCORES)], axis=0)
    return value, hidden_actor, np.asarray(rnn_hxs)
